# revision 1
# baseline (speedup 1.0000x reference)
"""Trainium2 Bass kernel for nn_Diffusion_GAT2 (gnn_message_passing).

Data-parallel over batch B=8 across 8 NeuronCores: each core processes one
batch element; the small weights are folded host-side and replicated.

Math (validated vs reference, see check_math.py):
  out = (diffusion(M4 @ x) + b4*S + conv_b) * emb + x    per batch element
where
  M4  = conv_w @ theta^T @ W_w          (all three 1x1 channel matmuls fold)
  b4  = conv_w @ theta^T @ W_b          (W_b pushed through the diffusion:
  S[m] = sum_n adj[n, m]                 contributes b4[e] * S[m])
  adj = topk-mask(softmax(e*cw + adj_f*cwa))  built from h = W_w @ sum_t(x)

Per-core pipeline:
  phase 1: stream x (fp32, cast to bf16 on DVE); per t-slice matmul with the
           x-slice as stationary operand produces z^T = (M4 @ x)^T directly
           in n-partition layout (z_r [n, (e,t)] bf16, SBUF-resident), and a
           second matmul on the same stationary accumulates h = W_w @ sum_t x
           on the PE (no DVE reduction).
  phase 2: adjacency fp32; fc scalars folded into cwa host-side so the
           combine chain runs on Pool; top-k via 13x max8+match_replace on
           negated post-softmax values (DVE).
  phase 3: diffusion psum[m,(e,t)] = sum_n adj[n,m] z[e,n,t] (adj bf16
           stationary), + (b4*S + conv_b) via identity-broadcast matmul;
           *emb via per-e-column ACT scale-copies; +x skip via re-read,
           added alternately on Pool/DVE; DMA out.
"""

import numpy as np

B, C, N, T = 8, 128, 512, 64
NCH = N // 128            # 4 n-chunks
KDROP = N - int(N * 0.8)  # 103 entries zapped per row
EBLK = 16                 # e-range per phase-3 column block
NCB = C // EBLK           # 16 column blocks
TB = 8                    # t-batch for phase-1 psum->sbuf copies

_CACHE = {}


def build_program(h_on_pe=True, tb=TB, ph3_transpose=True):
    """Build the Bass program (shared SPMD across the 8 cores)."""
    import concourse.bass as bass
    import concourse.bacc as bacc
    import concourse.mybir as mybir
    import concourse.tile as tile
    from contextlib import ExitStack

    f32 = mybir.dt.float32
    bf16 = mybir.dt.bfloat16
    Alu = mybir.AluOpType
    Act = mybir.ActivationFunctionType
    X = mybir.AxisListType.X

    nc = bacc.Bacc("TRN2", target_bir_lowering=False, debug=False)

    x_d = nc.dram_tensor("x", [C, N, T], f32, kind="ExternalInput")
    xb_d = nc.dram_tensor("xb", [C, N, T], bf16, kind="ExternalInput")
    WwTb_d = nc.dram_tensor("WwTb", [C, C], bf16, kind="ExternalInput")
    M4Tb_d = nc.dram_tensor("M4Tb", [C, C], bf16, kind="ExternalInput")
    Tb64_d = nc.dram_tensor("Tb64", [C, 1], f32, kind="ExternalInput")
    memT_d = nc.dram_tensor("memT", [C, N], f32, kind="ExternalInput")
    a1_d = nc.dram_tensor("a1", [C, 1], f32, kind="ExternalInput")
    a2_d = nc.dram_tensor("a2", [C, 1], f32, kind="ExternalInput")
    b4r_d = nc.dram_tensor("b4r", [1, C], f32, kind="ExternalInput")
    cbr_d = nc.dram_tensor("cbr", [1, C], f32, kind="ExternalInput")
    cw_d = nc.dram_tensor("cw", [N, N], f32, kind="ExternalInput")
    cwa00_d = nc.dram_tensor("cwa00", [N, N], bf16, kind="ExternalInput")
    cwa01_d = nc.dram_tensor("cwa01", [N, N], bf16, kind="ExternalInput")
    cwab_d = nc.dram_tensor("cwab", [N, N], bf16, kind="ExternalInput")
    embT_d = nc.dram_tensor("embT", [N, C], f32, kind="ExternalInput")
    identb_d = nc.dram_tensor("identb", [C, C], bf16, kind="ExternalInput")
    identf_d = nc.dram_tensor("identf", [C, C], f32, kind="ExternalInput")
    out_d = nc.dram_tensor("out", [C, N, T], f32, kind="ExternalOutput")

    scale = 1.0 / float(np.sqrt(np.float32(C)))

    with tile.TileContext(nc) as tc, ExitStack() as ctx:
        const = ctx.enter_context(tc.tile_pool(name="const", bufs=1))
        persist = ctx.enter_context(tc.tile_pool(name="persist", bufs=1))
        small = ctx.enter_context(tc.tile_pool(name="small", bufs=1))

        # ---------------- constants ----------------
        def cload(name, shape, dt, src):
            t_ = const.tile(shape, dt, tag=name, name=name)
            nc.sync.dma_start(t_, src)
            return t_

        WwTb = cload("WwTb", [C, C], bf16, WwTb_d[:])
        M4Tb = cload("M4Tb", [C, C], bf16, M4Tb_d[:])
        Tb64 = cload("Tb64", [C, 1], f32, Tb64_d[:])
        memT = cload("memT", [C, N], f32, memT_d[:])
        a1 = cload("a1", [C, 1], f32, a1_d[:])
        a2 = cload("a2", [C, 1], f32, a2_d[:])
        b4r = cload("b4r", [1, C], f32, b4r_d[:])
        cbr = cload("cbr", [1, C], f32, cbr_d[:])
        identb = cload("identb", [C, C], bf16, identb_d[:])
        identf = cload("identf", [C, C], f32, identf_d[:])
        cw_s, cwa00_s, cwa01_s, cwab_s, embT_s = [], [], [], [], []
        for ic in range(NCH):
            rsl = slice(ic * 128, (ic + 1) * 128)
            cw_s.append(cload(f"cw{ic}", [128, N], f32, cw_d[rsl, :]))
            cwa00_s.append(cload(f"cwa00{ic}", [128, N], bf16, cwa00_d[rsl, :]))
            cwa01_s.append(cload(f"cwa01{ic}", [128, N], bf16, cwa01_d[rsl, :]))
            cwab_s.append(cload(f"cwab{ic}", [128, N], bf16, cwab_d[rsl, :]))
            embT_s.append(cload(f"embT{ic}", [128, C], f32, embT_d[rsl, :]))
        ones_row = const.tile([1, N], f32, tag="ones_row")
        nc.vector.memset(ones_row, 1.0)
        ones_colb = const.tile([C, 1], bf16, tag="ones_colb")
        nc.vector.memset(ones_colb, 1.0)

        # ---------------- persistent state ----------------
        hT = persist.tile([C, N], f32, tag="hT")
        z_r = [
            persist.tile([128, C, T], bf16, tag=f"zr{i}", name=f"zr{i}")
            for i in range(NCH)
        ]
        adjb = [
            persist.tile([128, N], bf16, tag=f"adjb{i}", name=f"adjb{i}")
            for i in range(NCH)
        ]
        T2s = persist.tile([C, N], bf16, tag="T2s")

        # ------- phase 1: z^T (bf16, n-partition) and h via PE -------
        sxb = persist.tile([C, N], bf16, tag="sxb")
        with (
            tc.tile_pool(name="xb", bufs=2) as xbp,
            tc.tile_pool(name="hsb", bufs=2) as hsb,
            tc.tile_pool(name="ps1", bufs=3, space=bass.MemorySpace.PSUM) as ps1,
            tc.tile_pool(name="ps1h", bufs=1, space=bass.MemorySpace.PSUM) as ps1h,
        ):
            for ic in range(NCH):
                xbf = xbp.tile([C, 128, T], bf16, tag="xb")
                for q in range(2):
                    n0 = ic * 128 + q * 64
                    nc.sync.dma_start(
                        xbf[:, q * 64 : (q + 1) * 64, :], xb_d[:, n0 : n0 + 64, :]
                    )
                if h_on_pe:
                    hp = ps1h.tile([128, C], f32, tag="hp")
                if not h_on_pe:
                    with nc.allow_low_precision(reason="h from bf16 x by design"):
                        nc.vector.tensor_reduce(
                            sxb[:, ic * 128 : (ic + 1) * 128],
                            xbf,
                            axis=X,
                            op=Alu.add,
                        )
                for tbi in range(T // tb):
                    zp = ps1.tile([128, tb, C], f32, tag="zp")
                    for j in range(tb):
                        t = tbi * tb + j
                        nc.tensor.matmul(zp[:, j, :], lhsT=xbf[:, :, t], rhs=M4Tb)
                        if h_on_pe:
                            nc.tensor.matmul(
                                hp,
                                lhsT=xbf[:, :, t],
                                rhs=WwTb,
                                start=(t == 0),
                                stop=(t == T - 1),
                            )
                    if tbi % 2 == 0:
                        nc.scalar.activation(
                            z_r[ic][:, :, tbi * tb : (tbi + 1) * tb],
                            zp.rearrange("p t e -> p e t"),
                            Act.Copy,
                        )
                    else:
                        nc.vector.tensor_copy(
                            z_r[ic][:, :, tbi * tb : (tbi + 1) * tb],
                            zp.rearrange("p t e -> p e t"),
                        )
                if h_on_pe:
                    # h chunk [n, c] -> transpose to hT[:, chunk], add 64*W_b
                    hsb_t = hsb.tile([128, C], f32, tag="hsb")
                    nc.scalar.activation(hsb_t, hp, Act.Copy)
                    htp = ps1h.tile([C, 128], f32, tag="htp")
                    nc.tensor.transpose(htp, hsb_t, identf)
                    nc.scalar.activation(
                        hT[:, ic * 128 : (ic + 1) * 128], htp, Act.Identity, bias=Tb64
                    )
            if not h_on_pe:
                hp2 = ps1h.tile([C, N], f32, tag="hp2")
                nc.tensor.matmul(hp2, lhsT=WwTb, rhs=sxb)
                nc.vector.tensor_scalar(hT, hp2, Tb64, None, op0=Alu.add)

        # ---------------- phase 2: adjacency ----------------
        with (
            tc.tile_pool(name="wk", bufs=2) as wk,
            tc.tile_pool(name="st", bufs=2) as st,
            tc.tile_pool(name="ps2", bufs=2, space=bass.MemorySpace.PSUM) as ps2,
        ):
            w2p = ps2.tile([1, N], f32, tag="pbig")
            nc.tensor.matmul(w2p, lhsT=a2, rhs=hT)
            Wh2T = small.tile([1, N], f32, tag="Wh2T")
            nc.vector.tensor_copy(Wh2T, w2p)
            ones1c = small.tile([1, C], f32, tag="ones1c")
            nc.vector.memset(ones1c, 1.0)

            for ic in range(NCH):
                sl = slice(ic * 128, (ic + 1) * 128)
                w1p = ps2.tile([128, 1], f32, tag="pbig")
                nc.tensor.matmul(w1p, lhsT=hT[:, sl], rhs=a1)
                Wh1 = st.tile([128, 1], f32, tag="Wh1")
                nc.vector.tensor_copy(Wh1, w1p)

                # adj1 = softmax(relu(hT^T @ memT * scale))   (in-place chain)
                s1p = ps2.tile([128, N], f32, tag="pbig")
                nc.tensor.matmul(s1p, lhsT=hT[:, sl], rhs=memT)
                a1t = wk.tile([128, N], f32, tag="a1t")
                nc.scalar.activation(a1t, s1p, Act.Relu, scale=scale)
                mx = st.tile([128, 1], f32, tag="mx")
                nc.vector.tensor_reduce(mx, a1t, axis=X, op=Alu.max)
                nmx = st.tile([128, 1], f32, tag="nmx")
                nc.vector.tensor_scalar_mul(nmx, mx, -1.0)
                sm = st.tile([128, 1], f32, tag="sm")
                nc.scalar.activation(a1t, a1t, Act.Exp, bias=nmx, accum_out=sm)
                rc = st.tile([128, 1], f32, tag="rc")
                nc.vector.reciprocal(rc, sm)
                nc.vector.tensor_scalar_mul(a1t, a1t, rc)

                # adj2 = softmax(relu(hT^T @ hT * scale))
                s2p = ps2.tile([128, N], f32, tag="pbig")
                nc.tensor.matmul(s2p, lhsT=hT[:, sl], rhs=hT)
                a2t = wk.tile([128, N], f32, tag="a2t")
                nc.scalar.activation(a2t, s2p, Act.Relu, scale=scale)
                mx2 = st.tile([128, 1], f32, tag="mx")
                nc.vector.tensor_reduce(mx2, a2t, axis=X, op=Alu.max)
                nmx2 = st.tile([128, 1], f32, tag="nmx")
                nc.vector.tensor_scalar_mul(nmx2, mx2, -1.0)
                sm2 = st.tile([128, 1], f32, tag="sm")
                nc.scalar.activation(a2t, a2t, Act.Exp, bias=nmx2, accum_out=sm2)
                rc2 = st.tile([128, 1], f32, tag="rc")
                nc.vector.reciprocal(rc2, sm2)
                nc.vector.tensor_scalar_mul(a2t, a2t, rc2)

                # aw = (Wh1 + Wh2^T)*cw + adj1*cwa*fc00 + adj2*cwa*fc01 + cwa*fcb
                ep = ps2.tile([128, N], f32, tag="pbig")
                nc.tensor.matmul(ep, lhsT=ones1c, rhs=Wh2T)
                u = wk.tile([128, N], f32, tag="u")
                nc.vector.scalar_tensor_tensor(
                    u, ep, Wh1, cw_s[ic], op0=Alu.add, op1=Alu.mult
                )
                q1 = wk.tile([128, N], f32, tag="q1")
                nc.gpsimd.tensor_mul(q1, a1t, cwa00_s[ic])
                q2 = wk.tile([128, N], f32, tag="q2")
                nc.gpsimd.tensor_mul(q2, a2t, cwa01_s[ic])
                nc.gpsimd.tensor_add(q1, q1, q2)
                nc.gpsimd.tensor_add(q1, q1, cwab_s[ic])
                nc.gpsimd.tensor_add(u, u, q1)

                # neg = -softmax(u)
                mxw = st.tile([128, 1], f32, tag="mx")
                nc.vector.tensor_reduce(mxw, u, axis=X, op=Alu.max)
                nmxw = st.tile([128, 1], f32, tag="nmx")
                nc.vector.tensor_scalar_mul(nmxw, mxw, -1.0)
                smw = st.tile([128, 1], f32, tag="sm")
                exw = wk.tile([128, N], f32, tag="exw")
                nc.scalar.activation(exw, u, Act.Exp, bias=nmxw, accum_out=smw)
                rcw = st.tile([128, 1], f32, tag="rc")
                nc.vector.reciprocal(rcw, smw)
                nc.vector.tensor_scalar(
                    exw, exw, rcw, -1.0, op0=Alu.mult, op1=Alu.mult
                )

                # zap the KDROP smallest adj entries (= largest of neg)
                mxv = st.tile([128, 8], f32, tag="mxv")
                full_iters = KDROP // 8
                rem = KDROP - full_iters * 8
                for it in range(full_iters + (1 if rem else 0)):
                    nc.vector.max(mxv, exw)
                    if it == full_iters and rem:
                        nc.vector.memset(mxv[:, rem:8], 1.0)
                    nc.vector.match_replace(exw, mxv, exw, imm_value=-2.0)
                msk = wk.tile([128, N], f32, tag="msk")
                nc.vector.tensor_scalar(msk, exw, -1.5, None, op0=Alu.is_gt)
                nc.vector.scalar_tensor_tensor(
                    adjb[ic], exw, -1.0, msk, op0=Alu.mult, op1=Alu.mult
                )

            # S[m] = sum_n adj[n, m];  T2[e, m] = b4[e]*S[m] + conv_b[e]
            Sp = ps2.tile([1, N], f32, tag="pbig")
            for ic in range(NCH):
                nc.tensor.matmul(
                    Sp,
                    lhsT=ones_colb,
                    rhs=adjb[ic],
                    start=(ic == 0),
                    stop=(ic == NCH - 1),
                )
            Srow = small.tile([1, N], f32, tag="Srow")
            nc.vector.tensor_copy(Srow, Sp)
            T2p = ps2.tile([C, N], f32, tag="pbig")
            nc.tensor.matmul(T2p, lhsT=b4r, rhs=Srow, start=True, stop=False)
            nc.tensor.matmul(T2p, lhsT=cbr, rhs=ones_row, start=False, stop=True)
            nc.vector.tensor_copy(T2s, T2p)

        # ---------------- phase 3: diffusion + merge + skip ----------------
        if ph3_transpose:
            # psum [m,(e,t)] blocks -> *embT (DVE) -> staged [m,(e,th)] -> PE
            # back-transpose per t-slice -> og [e,(m,t)] -> +x via contiguous
            # accumulating DMA -> contiguous DMA out.
            TH = T // 2
            with (
                tc.tile_pool(name="ofh", bufs=2) as ofhp,
                tc.tile_pool(name="og", bufs=2) as ogp,
                tc.tile_pool(name="ps3", bufs=4, space=bass.MemorySpace.PSUM) as ps3,
                tc.tile_pool(name="pst", bufs=3, space=bass.MemorySpace.PSUM) as pst,
            ):
                for mc in range(NCH):
                    msl = slice(mc * 128, (mc + 1) * 128)
                    og = ogp.tile([C, 128, T], f32, tag="og")    # [e, (m, t)]
                    for th in range(2):
                        tsl = slice(th * TH, (th + 1) * TH)
                        ofh = ofhp.tile([128, C, TH], f32, tag="ofh")
                        for cb in range(NCB):
                            esl = slice(cb * EBLK, (cb + 1) * EBLK)
                            p3 = ps3.tile([128, EBLK, TH], f32, tag="p3")
                            for ic in range(NCH):
                                nc.tensor.matmul(
                                    p3,
                                    lhsT=adjb[ic][:, msl],
                                    rhs=z_r[ic][:, esl, tsl],
                                    start=(ic == 0),
                                    stop=False,
                                )
                            nc.tensor.matmul(
                                p3,
                                lhsT=T2s[:, msl],
                                rhs=identb[:, esl].to_broadcast([C, EBLK, TH]),
                                start=False,
                                stop=True,
                            )
                            nc.vector.tensor_mul(
                                ofh[:, esl, :],
                                p3,
                                embT_s[mc][:, esl].to_broadcast([128, EBLK, TH]),
                            )
                        for tg in range(TH // 4):
                            tp4 = pst.tile([C, 4, 128], f32, tag="tp")
                            for j in range(4):
                                nc.tensor.transpose(
                                    tp4[:, j, :], ofh[:, :, tg * 4 + j], identf
                                )
                            t0 = th * TH + tg * 4
                            nc.scalar.activation(
                                og[:, :, t0 : t0 + 4],
                                tp4.rearrange("p j m -> p m j"),
                                Act.Copy,
                            )
                    # skip: og += x[:, msl, :] via contiguous accumulating
                    # DMAs (split into quarters: SWDGE accum >8KB/partition
                    # overflows the descriptor ring and wedges the device)
                    for qm in range(4):
                        qsl = slice(mc * 128 + qm * 32, mc * 128 + (qm + 1) * 32)
                        nc.gpsimd.dma_start(
                            og[:, qm * 32 : (qm + 1) * 32, :],
                            x_d[:, qsl, :],
                            accum_op=Alu.add,
                        )
                    nc.sync.dma_start(out_d[:, msl, :], og)
        else:
            # v4-style: direct strided writes, xs re-read, Pool adds
            with (
                tc.tile_pool(name="of", bufs=4) as ofp,
                tc.tile_pool(name="xs", bufs=4) as xsp,
                tc.tile_pool(name="ps3", bufs=4, space=bass.MemorySpace.PSUM) as ps3,
            ):
                EB2, NB2 = 8, 16
                for mc in range(NCH):
                    msl = slice(mc * 128, (mc + 1) * 128)
                    for cb in range(NB2):
                        esl = slice(cb * EB2, (cb + 1) * EB2)
                        p3 = ps3.tile([128, EB2, T], f32, tag="p3")
                        for ic in range(NCH):
                            nc.tensor.matmul(
                                p3,
                                lhsT=adjb[ic][:, msl],
                                rhs=z_r[ic][:, esl, :],
                                start=(ic == 0),
                                stop=False,
                            )
                        nc.tensor.matmul(
                            p3,
                            lhsT=T2s[:, msl],
                            rhs=identb[:, esl].to_broadcast([C, EB2, T]),
                            start=False,
                            stop=True,
                        )
                        of = ofp.tile([128, EB2, T], f32, tag="of")
                        nc.vector.tensor_mul(
                            of,
                            p3,
                            embT_s[mc][:, esl].to_broadcast([128, EB2, T]),
                        )
                        xs = xsp.tile([128, EB2, T], f32, tag="xs")
                        src = x_d[esl, msl, :].rearrange("e n t -> n e t")
                        nc.sync.dma_start(xs, src)
                        nc.gpsimd.tensor_add(of, of, xs)
                        dst = out_d[esl, msl, :].rearrange("e n t -> n e t")
                        nc.sync.dma_start(dst, of)

    nc.compile()
    return nc


def _host_prep(inputs):
    """Fold the small channel matmuls and lay out replicated weights."""
    import ml_dtypes

    f = np.float32
    bf = ml_dtypes.bfloat16
    W_w = np.asarray(inputs["W_w"], f)
    W_b = np.asarray(inputs["W_b"], f)
    conv_w = np.asarray(inputs["conv_w"], f)
    conv_b = np.asarray(inputs["conv_b"], f)
    theta = np.asarray(inputs["theta"], f)
    memory = np.asarray(inputs["memory"], f)
    a_vec = np.asarray(inputs["a_vec"], f)
    cw = np.asarray(inputs["cw"], f)
    cwa = np.asarray(inputs["cwa"], f)
    fc_w = np.asarray(inputs["fc_w"], f)
    fc_b = np.asarray(inputs["fc_b"], f)
    emb = np.asarray(inputs["emb"], f)

    M2T = theta @ conv_w.T
    M4T = W_w.T @ M2T
    b4 = M2T.T @ W_b
    common = {
        "WwTb": np.ascontiguousarray(W_w.T).astype(bf),
        "M4Tb": np.ascontiguousarray(M4T).astype(bf),
        "Tb64": np.ascontiguousarray((T * W_b).reshape(C, 1)),
        "memT": np.ascontiguousarray(memory.T),
        "a1": np.ascontiguousarray(a_vec[:C]),
        "a2": np.ascontiguousarray(a_vec[C:]),
        "b4r": np.ascontiguousarray(b4.reshape(1, C)),
        "cbr": np.ascontiguousarray(conv_b.reshape(1, C)),
        "cw": cw,
        "cwa00": (cwa * fc_w[0, 0]).astype(bf),
        "cwa01": (cwa * fc_w[0, 1]).astype(bf),
        "cwab": (cwa * fc_b[0]).astype(bf),
        "embT": np.ascontiguousarray(emb[0, :, :, 0].T),
        "identb": np.eye(C, dtype=bf),
        "identf": np.eye(C, dtype=f),
    }
    x = np.asarray(inputs["x"], f)
    in_maps = [
        dict(
            common,
            x=np.ascontiguousarray(x[b]),
            xb=np.ascontiguousarray(x[b]).astype(bf),
        )
        for b in range(B)
    ]
    return in_maps


def get_runner():
    """Build (once) a persistently-jitted SPMD callable in_maps -> results."""
    key = "runner"
    if key not in _CACHE:
        import jax
        from jax.sharding import Mesh, PartitionSpec
        from jax.experimental.shard_map import shard_map
        import concourse.mybir as mybir
        from concourse import bass2jax

        bass2jax.install_neuronx_cc_hook()
        nc = build_program()

        part_name = nc.partition_id_tensor.name if nc.partition_id_tensor else None
        in_names, out_names, out_avals = [], [], []
        for alloc in nc.m.functions[0].allocations:
            if not isinstance(alloc, mybir.MemoryLocationSet):
                continue
            name = alloc.memorylocations[0].name
            if alloc.kind == "ExternalInput":
                if name != part_name:
                    in_names.append(name)
            elif alloc.kind == "ExternalOutput":
                out_names.append(name)
                out_avals.append(
                    jax.core.ShapedArray(
                        tuple(alloc.tensor_shape), mybir.dt.np(alloc.dtype)
                    )
                )
        n_params = len(in_names)
        all_names = in_names + out_names
        if part_name is not None:
            all_names = all_names + [part_name]

        def _body(*args):
            operands = list(args)
            if part_name is not None:
                operands.append(bass2jax.partition_id_tensor())
            outs = bass2jax._bass_exec_p.bind(
                *operands,
                out_avals=tuple(out_avals),
                in_names=tuple(all_names),
                out_names=tuple(out_names),
                lowering_input_output_aliases=(),
                sim_require_finite=True,
                sim_require_nnan=True,
                nc=nc,
            )
            return tuple(outs)

        devices = jax.devices()[:B]
        mesh = Mesh(np.array(devices), ("core",))
        n_outs = len(out_names)
        sharded = jax.jit(
            shard_map(
                _body,
                mesh=mesh,
                in_specs=(PartitionSpec("core"),) * (n_params + n_outs),
                out_specs=(PartitionSpec("core"),) * n_outs,
                check_rep=False,
            ),
            donate_argnums=tuple(range(n_params, n_params + n_outs)),
            keep_unused=True,
        )

        def run(in_maps, timing_iters=0):
            concat_in = [
                np.concatenate([np.asarray(m[nm]) for m in in_maps], axis=0)
                for nm in in_names
            ]
            zeros = [
                np.zeros((B * av.shape[0], *av.shape[1:]), av.dtype)
                for av in out_avals
            ]
            out_arrs = sharded(*concat_in, *zeros)
            jax.block_until_ready(out_arrs)
            if timing_iters:
                import time
                from jax.sharding import NamedSharding

                sh = NamedSharding(mesh, PartitionSpec("core"))
                dev_in = [jax.device_put(a, sh) for a in concat_in]
                zsets = [
                    [
                        jax.device_put(
                            np.zeros((B * av.shape[0], *av.shape[1:]), av.dtype), sh
                        )
                        for av in out_avals
                    ]
                    for _ in range(timing_iters)
                ]
                jax.block_until_ready(dev_in)
                jax.block_until_ready(zsets)
                times = []
                for i in range(timing_iters):
                    t0 = time.perf_counter()
                    r = sharded(*dev_in, *zsets[i])
                    jax.block_until_ready(r)
                    times.append(time.perf_counter() - t0)
                run.last_times = times
            return [
                {
                    nm: np.asarray(out_arrs[i]).reshape(B, *out_avals[i].shape)[c]
                    for i, nm in enumerate(out_names)
                }
                for c in range(B)
            ]

        _CACHE[key] = run
    return _CACHE[key]


def kernel(**inputs) -> np.ndarray:
    in_maps = _host_prep(inputs)
    run = get_runner()
    results = run(in_maps)
    return np.stack([results[b]["out"] for b in range(B)], axis=0)

